# revision 10
# baseline (speedup 1.0000x reference)
"""Trainium2 Bass kernel for nn_CustomCausalAttention (sparse block-causal
attention, D=1024, 16 heads x 64, L=1152 = 1024 pc + 128 joint tokens).

Sharding: 2 heads per core across 8 cores (tensor parallel on the head dim of
w_qkv and the input dim of w_fc). Each core computes its 2 heads' attention and
a partial FC output [1024, 1152]; the host sums the 8 partials.

Device-side layout (everything transposed so matmuls contract on partitions):
  - xT [D, L] f32r in SBUF (8 K-tiles of [128, L])
  - qT/kT per head [64, L] f32r (from QKV matmuls, PSUM split-copied)
  - scores computed TRANSPOSED: ST[j, i] = k-tile.T @ q-chunk (per head)
  - softmax without max subtraction (scores ~ N(0,1), fp32 exp is safe);
    the normalizer Z is obtained by appending a ones-column to V (PV matmul
    row 64 = sum_j E[j,i]), then out = PV[0:64] * broadcast(1/Z) where the
    broadcast across partitions is a K=1 ones matmul.
  - V in natural [j, dh] layout WITHOUT PE transposes (is_transpose matmuls
    corrupt subsequent f32r matmuls on this toolchain): vT is split into
    bf16 hi + lo halves, each DMA-xbar-transposed (16-bit only), recombined
    with a DVE add into f32r V tiles. Exact to ~2^-16.
  - fc: yT_partial[e, i] = wfcT.T @ outT, K=128 per-core head dims.

All matmuls use dtype float32r (TRN2 4x-rate fp32 "transpose mode",
~1e-4 rounding) with uniform tile_position (0,0).

Walrus-compat patches (this neuronxcc build allows only ONE sync-wait per
instruction): the Tile tail drain and any instruction with >1 waits get the
excess waits moved onto same-engine nops inserted immediately before them.
"""

import numpy as np
import ml_dtypes

import concourse.bass as bass
import concourse.mybir as mybir
import concourse.tile as tile
from concourse.bass_utils import run_bass_kernel_spmd
from bass_rust import ScopedClock

F32 = mybir.dt.float32
F32R = mybir.dt.float32r
BF16 = mybir.dt.bfloat16
EXP = mybir.ActivationFunctionType.Exp

L = 1152          # sequence (1024 pc + 128 joint)
D = 1024
DH = 64
N_CORES = 8
HPC = 2           # heads per core
NPC = 1024
NKT = D // 128    # 8 K tiles of the contraction dim
NJT = L // 128    # 9 key tiles
CHUNKS = [(0, 512), (512, 384), (896, 256)]  # query chunks (start, size)
SCALE = 1.0 / np.sqrt(DH)
V_MODE = "f32r"  # or "bf16"

# ---------------------------------------------------------------------------
# walrus-compat patches
# ---------------------------------------------------------------------------

_drain_patched = False


def _patch_tile_drain():
    global _drain_patched
    if _drain_patched:
        return
    _drain_patched = True

    def _drain_and_barrier(self, tick_clock, wait_clock):
        nc = self.nc
        drain_bi = nc.sync.drain()
        wait_clock.add_sem_waits(drain_bi.ins, ScopedClock({None: tick_clock.global_clock}))
        inst = drain_bi.ins
        si = inst.sync_info
        waits = list((si.on_wait if si else None) or [])
        if len(waits) > 1:
            bb = nc.cur_bb.bb
            assert bb.instructions[-1].name == inst.name
            bb.instructions.pop()
            for ch in waits[:-1]:
                nop = nc.sync.nop(nofuse=True, hint="drain_split")
                nop.ins.sync_info = mybir.SyncInfo(on_wait=[ch], on_update=[])
            inst.sync_info = mybir.SyncInfo(
                on_wait=[waits[-1]],
                on_update=list((si.on_update if si else None) or []),
            )
            bb.add_instruction(inst)
        nc.all_engine_barrier()
        popped = nc._tile_sem_poison_stack.pop()
        assert popped is self._sem_poison
        nc.clear_and_free_semaphores(list(self.sems.allocated().values()))
        nc.all_engine_barrier()

    tile.TileContext._drain_and_barrier = _drain_and_barrier


def _split_excess_waits(nc, max_waits=1):
    for f in nc.m.functions:
        for bb in f.blocks:
            new = []
            for inst in bb.instructions:
                si = inst.sync_info
                waits = list((si.on_wait if si else None) or [])
                if len(waits) > max_waits:
                    for ch in waits[:-max_waits]:
                        nop_bi = nc.engines[inst.engine].nop(nofuse=True, hint="wait_split")
                        nop = nop_bi.ins
                        cb = nc.cur_bb.bb
                        assert cb.instructions[-1].name == nop.name
                        cb.instructions.pop()
                        nop.sync_info = mybir.SyncInfo(on_wait=[ch], on_update=[])
                        new.append(nop)
                    inst.sync_info = mybir.SyncInfo(
                        on_wait=waits[-max_waits:],
                        on_update=list((si.on_update if si else None) or []),
                    )
                new.append(inst)
            bb.instructions[:] = new


# ---------------------------------------------------------------------------
# device program
# ---------------------------------------------------------------------------

def build_program(debug_taps=False):
    _patch_tile_drain()
    nc = bass.Bass()

    xT = nc.dram_tensor("xT", [D, L], F32R, kind="ExternalInput")
    wqkvT = nc.dram_tensor("wqkvT", [D, 3 * 128], F32R, kind="ExternalInput")
    wfcT = nc.dram_tensor("wfcT", [128, D], F32R, kind="ExternalInput")
    tri = nc.dram_tensor("tri", [128, 256], F32R, kind="ExternalInput")
    yT = nc.dram_tensor("yT", [D, L], F32, kind="ExternalOutput")
    if debug_taps:
        dbg_q0 = nc.dram_tensor("dbg_q0", [DH, L], F32R, kind="ExternalOutput")
        dbg_k0 = nc.dram_tensor("dbg_k0", [DH, L], F32R, kind="ExternalOutput")
        dbg_vp0 = nc.dram_tensor("dbg_vp0", [128, NJT * (DH + 1)], F32R, kind="ExternalOutput")
        dbg_outT = nc.dram_tensor("dbg_outT", [128, L], F32R, kind="ExternalOutput")

    with tile.TileContext(nc) as tc:
        with (
            tc.tile_pool(name="sbx", bufs=1) as sbx,        # xT tiles (persistent)
            tc.tile_pool(name="sbw", bufs=1) as sbw,        # weights / constants
            tc.tile_pool(name="sbqk", bufs=1) as sbqk,      # qT/kT/Vp/outT persistents
            tc.tile_pool(name="sbv", bufs=1) as sbv,        # V transpose staging
            tc.tile_pool(name="sbe", bufs=4) as sbe,        # E tiles
            tc.tile_pool(name="sbz", bufs=2) as sbz,        # reciprocal rows, om
            tc.tile_pool(name="sby", bufs=3) as sby,        # fc output staging
            tc.tile_pool(name="psmm", bufs=2, space="PSUM") as psmm,
            tc.tile_pool(name="pssc", bufs=2, space="PSUM") as pssc,
            tc.tile_pool(name="pspv", bufs=2, space="PSUM") as pspv,
            tc.tile_pool(name="psbc", bufs=2, space="PSUM") as psbc,
        ):
            # ---------------- loads ----------------
            xt = [sbx.tile([128, L], F32R, name=f"x{k}", tag=f"x{k}") for k in range(NKT)]
            wt = [sbw.tile([128, 3 * 128], F32R, name=f"w{k}", tag=f"w{k}") for k in range(NKT)]
            for k in range(NKT):
                nc.sync.dma_start(out=wt[k], in_=wqkvT[k * 128:(k + 1) * 128, :])
            wf = sbw.tile([128, D], F32R, name="wf", tag="wf")
            nc.sync.dma_start(out=wf, in_=wfcT[:, :])
            trit = sbw.tile([128, 256], F32R, name="tri", tag="tri")
            nc.sync.dma_start(out=trit, in_=tri[:, :])
            onecol = sbw.tile([128, 1], F32, name="onecol", tag="onecol")
            nc.vector.memset(onecol, 1.0)
            ones64f = sbw.tile([1, DH], F32, name="ones64f", tag="ones64f")
            nc.vector.memset(ones64f, 1.0)
            ones64 = sbw.tile([1, DH], F32R, name="ones64", tag="ones64")
            nc.vector.tensor_copy(ones64, ones64f)

            # x loaded in L-chunk order so early query chunks unblock sooner
            for (c0, cn) in CHUNKS:
                for k in range(NKT):
                    nc.sync.dma_start(
                        out=xt[k][:, c0:c0 + cn],
                        in_=xT[k * 128:(k + 1) * 128, c0:c0 + cn],
                    )

            # ---------------- qkv projections ----------------
            qh = [sbqk.tile([DH, L], F32R, name=f"q{h}", tag=f"q{h}") for h in range(HPC)]
            kh = [sbqk.tile([DH, L], F32R, name=f"k{h}", tag=f"k{h}") for h in range(HPC)]
            for (c0, cn) in CHUNKS:
                for p, dsts in ((0, qh), (1, kh)):
                    ps = psmm.tile([128, cn], F32, name="mm", tag="mm")
                    for k in range(NKT):
                        nc.tensor.matmul(
                            ps,
                            lhsT=wt[k][:, p * 128:(p + 1) * 128],
                            rhs=xt[k][:, c0:c0 + cn],
                            start=(k == 0), stop=(k == NKT - 1),
                        )
                    nc.vector.tensor_copy(dsts[0][:, c0:c0 + cn], ps[0:DH, :])
                    nc.vector.tensor_copy(dsts[1][:, c0:c0 + cn], ps[DH:128, :])

            # ---------------- V in natural [j, dh] layout ------------------
            # out[l, m] = sum_d xT[d, l] * wvT[d, m]; lhsT = xT l-tile,
            # rhs = wv columns. N=128 (f32r 4 cyc/row) but avoids the broken
            # PE-transpose / DMA-xbar-transpose paths entirely.
            vp = [sbqk.tile([128, NJT * (DH + 1)], F32R, name=f"vp{h}", tag=f"vp{h}") for h in range(HPC)]
            if V_MODE == "bf16":
                xbf = [sbv.tile([128, L], BF16, name=f"xbf{k}", tag=f"xbf{k}") for k in range(NKT)]
                wvbf = [sbv.tile([128, 128], BF16, name=f"wvbf{k}", tag=f"wvbf{k}") for k in range(NKT)]
                with nc.allow_low_precision(reason="bf16 V projection"):
                    for k in range(NKT):
                        nc.scalar.copy(xbf[k], xt[k])
                        nc.vector.tensor_copy(wvbf[k], wt[k][:, 256:384])
            for lt in range(NJT):
                psv = psmm.tile([128, 128], F32, name="vn", tag="mm")
                for k in range(NKT):
                    if V_MODE == "bf16":
                        nc.tensor.matmul(
                            psv, lhsT=xbf[k][:, lt * 128:(lt + 1) * 128],
                            rhs=wvbf[k], start=(k == 0), stop=(k == NKT - 1),
                        )
                    else:
                        nc.tensor.matmul(
                            psv, lhsT=xt[k][:, lt * 128:(lt + 1) * 128],
                            rhs=wt[k][:, 256:384], start=(k == 0), stop=(k == NKT - 1),
                        )
                for h in range(HPC):
                    base = jt_base = lt * (DH + 1)
                    nc.vector.tensor_copy(vp[h][:, base:base + DH], psv[:, h * DH:(h + 1) * DH])
                    nc.vector.tensor_copy(vp[h][:, base + DH:base + DH + 1], onecol)

            # ---------------- attention (transposed scores) ----------------
            outT = sbqk.tile([128, L], F32R, name="outT", tag="outT")
            for h in range(HPC):
                for ci, (c0, cn) in enumerate(CHUNKS):
                    njt = NJT if ci == 2 else NJT - 1
                    po = pspv.tile([DH + 1, cn], F32, name="pv", tag="pv")
                    for jt in range(njt):
                        ps = pssc.tile([128, cn], F32, name="sc", tag="sc")
                        nc.tensor.matmul(
                            ps,
                            lhsT=kh[h][:, jt * 128:(jt + 1) * 128],
                            rhs=qh[h][:, c0:c0 + cn],
                            start=True, stop=True,
                        )
                        e = sbe.tile([128, cn], F32R, name="e", tag="e")
                        nc.scalar.activation(e, ps, EXP, scale=float(SCALE))
                        if jt == NJT - 1:
                            em = sbe.tile([128, cn], F32R, name="em", tag="em")
                            nc.vector.tensor_mul(em, e, trit)
                            e = em
                        nc.tensor.matmul(
                            po,
                            lhsT=vp[h][:, jt * (DH + 1):(jt + 1) * (DH + 1)],
                            rhs=e,
                            start=(jt == 0), stop=(jt == njt - 1),
                            skip_group_check=True,
                        )
                    # normalize: out = po[0:64] * bcast(1/Z), Z = po[64]
                    zr = sbz.tile([1, cn], F32R, name="zr", tag="zr")
                    with nc.allow_low_precision(reason="f32r is ~tf32; consistent with matmul dtype"):
                        nc.vector.reciprocal(zr, po[DH:DH + 1, :])
                    pb = psbc.tile([DH, cn], F32, name="bc", tag="bc")
                    nc.tensor.matmul(pb, lhsT=ones64, rhs=zr, start=True, stop=True)
                    pbs = sbz.tile([DH, cn], F32, name="pbs", tag="pbs")
                    nc.scalar.copy(pbs, pb)
                    om = sbz.tile([DH, cn], F32R, name="om", tag="om")
                    nc.vector.tensor_mul(om, po[0:DH, :], pbs)
                    nc.vector.tensor_copy(outT[h * DH:(h + 1) * DH, c0:c0 + cn], om)

            if debug_taps:
                nc.sync.dma_start(out=dbg_q0[:, :], in_=qh[0])
                nc.sync.dma_start(out=dbg_k0[:, :], in_=kh[0])
                nc.sync.dma_start(out=dbg_vp0[:, :], in_=vp[0])
                nc.sync.dma_start(out=dbg_outT[:, :], in_=outT)

            # ---------------- fc partial ----------------
            for et in range(NKT):
                ystage = sby.tile([128, L], F32, name="y", tag="y")
                for ci, (c0, cn) in enumerate(CHUNKS):
                    pf = psmm.tile([128, cn], F32, name="mm", tag="mm")
                    nc.tensor.matmul(
                        pf,
                        lhsT=wf[:, et * 128:(et + 1) * 128],
                        rhs=outT[:, c0:c0 + cn],
                        start=True, stop=True,
                    )
                    if ci == 0:
                        nc.scalar.copy(ystage[:, c0:c0 + cn], pf)
                    else:
                        nc.vector.tensor_copy(ystage[:, c0:c0 + cn], pf)
                nc.sync.dma_start(out=yT[et * 128:(et + 1) * 128, :], in_=ystage)

    _split_excess_waits(nc, 1)
    return nc


_prog = None


def _get_program():
    global _prog
    if _prog is None:
        _prog = build_program()
    return _prog


# ---------------------------------------------------------------------------
# host-side sharding / unsharding
# ---------------------------------------------------------------------------

def make_in_maps(x, w_qkv, w_fc):
    x2 = np.asarray(x, dtype=np.float32).reshape(L, D)
    xT = np.ascontiguousarray(x2.T)
    w_qkv = np.asarray(w_qkv, dtype=np.float32)
    w_fc = np.asarray(w_fc, dtype=np.float32)

    # joint-tile mask in transposed orientation: rows j (keys 1024+j),
    # cols 0:128 -> queries 896..1023 (pc never attends joints -> 0),
    # cols 128:256 -> queries 1024+ii, allowed iff j <= ii.
    trimask = np.zeros((128, 256), dtype=np.float32)
    jj = np.arange(128)[:, None]
    ii = np.arange(128)[None, :]
    trimask[:, 128:] = (jj <= ii).astype(np.float32)

    in_maps = []
    for c in range(N_CORES):
        g0, g1 = 2 * c, 2 * c + 1
        cols = []
        for blk in range(3):  # q, k, v
            for g in (g0, g1):
                rows = w_qkv[blk * D + g * DH: blk * D + (g + 1) * DH, :]
                cols.append(np.ascontiguousarray(rows.T))
        wqkvT_c = np.concatenate(cols, axis=1)  # [1024, 384]
        wfcT_c = np.ascontiguousarray(w_fc[:, c * 128:(c + 1) * 128].T)  # [128, 1024]
        in_maps.append({
            "xT": xT,
            "wqkvT": wqkvT_c,
            "wfcT": wfcT_c,
            "tri": trimask,
        })
    return in_maps


def combine_outputs(results):
    acc = np.zeros((D, L), dtype=np.float64)
    for r in results:
        acc += np.asarray(r["yT"], dtype=np.float64)
    return np.ascontiguousarray(acc.T).astype(np.float32).reshape(1, L, D)


def kernel(x, w_qkv, w_fc):
    nc = _get_program()
    in_maps = make_in_maps(x, w_qkv, w_fc)
    res = run_bass_kernel_spmd(nc, in_maps, core_ids=list(range(N_CORES)))
    return combine_outputs(res.results)
